# revision 5
# baseline (speedup 1.0000x reference)
"""Haar (db1) DWT kernel for Trainium2, 8-core data-parallel.

Computes, for x of shape (B=64, T=4096, C=128):
  approx  = (x[:, 0::2, :] + x[:, 1::2, :]) * 1/sqrt(2)      -> (B, T/2, C)
  details = transpose((xe - xo) * 1/sqrt(2), (0, 2, 1))       -> (B, C, T/2)
  r = 0.01*mean|details| + 0.01*|mean(approx) - mean(x)|      -> scalar

Sharding: pure data parallel over batch (8 batches per core). The only
cross-core term is the scalar mean reductions in r; each core emits
per-partition partial sums which are combined on the host.
"""

import sys

sys.path.insert(0, "/opt/trn_rl_repo")

from contextlib import ExitStack

import numpy as np

import concourse.bass as bass
import concourse.tile as tile
from concourse import bacc, mybir
from concourse.bass_utils import run_bass_kernel_spmd

INV_SQRT2 = 0.7071067811865476
REGU_DETAILS = 0.01
REGU_APPROX = 0.01

NCORES = 8
B, T, C = 64, 4096, 128
BP = B // NCORES  # batches per core
M = T // 256  # 128-partition chunks of output index i per batch (i = m*128 + p)
F32 = mybir.dt.float32


def build_nc(bp: int = BP) -> bass.Bass:
    nc = bacc.Bacc(
        "TRN2",
        target_bir_lowering=False,
        debug=False,
        num_devices=NCORES,
    )
    x = nc.dram_tensor("x", [bp, T, C], F32, kind="ExternalInput")
    ident = nc.dram_tensor("ident", [128, 128], F32, kind="ExternalInput")
    approx = nc.dram_tensor("approx", [bp, T // 2, C], F32, kind="ExternalOutput")
    detailsT = nc.dram_tensor("detailsT", [bp, C, T // 2], F32, kind="ExternalOutput")
    # Per-partition partial sums: sacc = sum of approx (scaled), racc = sum |details|.
    sacc = nc.dram_tensor("sacc", [128, bp], F32, kind="ExternalOutput")
    racc = nc.dram_tensor("racc", [128, bp], F32, kind="ExternalOutput")

    with tile.TileContext(nc) as tc:
        with ExitStack() as ctx:
            xpool = ctx.enter_context(tc.tile_pool(name="x", bufs=3))
            dpool = ctx.enter_context(tc.tile_pool(name="du", bufs=2))
            apool = ctx.enter_context(tc.tile_pool(name="appr", bufs=2))
            dTpool = ctx.enter_context(tc.tile_pool(name="dT", bufs=2))
            pspool = ctx.enter_context(tc.tile_pool(name="ps", bufs=4, space="PSUM"))
            cpool = ctx.enter_context(tc.tile_pool(name="const", bufs=1))

            ident_sb = cpool.tile([128, 128], F32)
            nc.sync.dma_start(ident_sb[:], ident[:])
            sacc_sb = cpool.tile([128, bp], F32, tag="sacc")
            racc_sb = cpool.tile([128, bp], F32, tag="racc")

            for b in range(bp):
                # x[b] viewed as t = m*256 + p*2 + e: partition p holds pair
                # (x[2i], x[2i+1]) for i = m*128 + p; 1 KiB contiguous chunks.
                xb = xpool.tile([128, M, 2, C], F32)
                nc.sync.dma_start(
                    xb[:], x[b].rearrange("(m p e) c -> p m e c", p=128, e=2)
                )
                xe = xb[:, :, 0, :]
                xo = xb[:, :, 1, :]

                # au = xe + xo (unscaled); ACT scales into appr and emits the
                # per-partition sum of the scaled values as accum_out.
                au = apool.tile([128, M, C], F32, tag="au")
                nc.vector.tensor_add(au[:], xe, xo)
                appr = apool.tile([128, M, C], F32, tag="appr")
                nc.scalar.activation(
                    appr[:],
                    au[:],
                    mybir.ActivationFunctionType.Copy,
                    scale=INV_SQRT2,
                    accum_out=sacc_sb[:, b : b + 1],
                )
                # du = xe - xo (unscaled; scale applied in the PSUM->SBUF copy
                # and on the host for the abs-sum partials).
                du = dpool.tile([128, M, C], F32)
                nc.vector.tensor_sub(du[:], xe, xo)
                nc.vector.tensor_reduce(
                    out=racc_sb[:, b : b + 1],
                    in_=du[:],
                    axis=mybir.AxisListType.XY,
                    op=mybir.AluOpType.add,
                    apply_absolute_value=True,
                )

                # Transpose each [i,c] 128x128 tile to [c,i] via the PE, four
                # per PSUM bank, then copy bank-wide to the [c, i] SBUF buffer.
                dT = dTpool.tile([128, T // 2], F32)
                for g in range(M // 4):
                    ps = pspool.tile([128, 512], F32)
                    for j in range(4):
                        m = 4 * g + j
                        nc.tensor.transpose(
                            ps[:, 128 * j : 128 * (j + 1)], du[:, m, :], ident_sb[:]
                        )
                    nc.scalar.mul(dT[:, 512 * g : 512 * (g + 1)], ps[:], INV_SQRT2)

                nc.sync.dma_start(detailsT[b], dT[:])
                nc.sync.dma_start(
                    approx[b].rearrange("(m p) c -> p m c", p=128), appr[:]
                )

            nc.sync.dma_start(sacc[:], sacc_sb[:])
            nc.sync.dma_start(racc[:], racc_sb[:])
    nc.compile()
    return nc


_NC_CACHE: dict[int, bass.Bass] = {}


def _get_nc(bp: int = BP) -> bass.Bass:
    if bp not in _NC_CACHE:
        _NC_CACHE[bp] = build_nc(bp)
    return _NC_CACHE[bp]


def _finish_scalar(sacc_total: float, racc_total: float) -> np.float32:
    n_half = B * (T // 2) * C  # elements in approx / details
    n_full = B * T * C
    rd = REGU_DETAILS * (racc_total / n_half)
    mean_approx = sacc_total / n_half
    mean_x = (sacc_total / INV_SQRT2) / n_full
    rc = REGU_APPROX * abs(mean_approx - mean_x)
    return np.float32(rd + rc)


def kernel(
    x: np.ndarray,
    _trace: bool = False,
    _results_out: dict | None = None,
    _tmpdir: str | None = None,
):
    x = np.asarray(x, dtype=np.float32)
    assert x.shape == (B, T, C), x.shape
    nc = _get_nc()
    ident = np.eye(128, dtype=np.float32)
    in_maps = [
        {"x": np.ascontiguousarray(x[i * BP : (i + 1) * BP]), "ident": ident}
        for i in range(NCORES)
    ]
    out = run_bass_kernel_spmd(
        nc, in_maps, list(range(NCORES)), trace=_trace, tmpdir=_tmpdir
    )
    if _results_out is not None:
        _results_out["bass_results"] = out
    res = out.results
    approx = np.concatenate([r["approx"] for r in res], axis=0)
    details = np.concatenate([r["detailsT"] for r in res], axis=0)
    sacc_total = float(sum(r["sacc"].sum(dtype=np.float64) for r in res))
    racc_total = float(sum(r["racc"].sum(dtype=np.float64) for r in res)) * INV_SQRT2
    r = _finish_scalar(sacc_total, racc_total)
    return approx, r, details
